# revision 1
# baseline (speedup 1.0000x reference)
"""DeepMemoryMachine Trainium2 Bass kernel (v2).

Model: 16384-step sequential GRU + discrete write-once memory:
    h_new = GRU(h_prev, x_t)
    q     = argmax(C_w @ h_new + C_b)          (512 addresses)
    hit (written[q] & q>0):   h_out = M[q]     (read replaces state)
    miss:                     h_out = h_new;  first-visit q>0 writes M[q]=h_new

Design notes
------------
* Memory rows are write-once, so W_hh @ M[q] is memoized per address in an
  SBUF table (GHtab, bias-free: b_hh is folded into the precomputed Gi).
  On ~97% of steps (read-hits) the recurrent matvec is a dynamic-slice
  gather keyed by the argmax index held in engine registers.
* The per-step critical chain is gather -> gates (DVE/ACT) -> logits
  matmul (PE, float32r: 1 cycle/row vs fp32's 4) -> max/max_index (DVE)
  -> register load of q -> next gather.
* DVE never branches: the miss path (write Mtab/GHtab/wflag) runs under
  tc.If on ACT+PE only.  h_out == Mtab[q] on BOTH paths because the miss
  body first copies h_new into Mtab[q] (address 0 acts as a scratch slot
  for the continuous state and is never flagged written).
* Everything runs on one NeuronCore, replicated SPMD on all 8 cores (the
  time loop is inherently sequential and there is a single sequence).

Layout: 512-vectors are SBUF [128, 4] with element (p, j) = v[p + 128*j];
1536-vectors are [128, 12] likewise.  Gi = X @ W_ih.T + b_ih + b_hh is
precomputed on-device into HBM as GiT[12, 128, T] and streamed per
512-step chunk.  H (the h_out history) is stored transposed Ht[512, T]
and the final Y = H @ V_w.T + V_b is a dense matmul pass.
"""

import numpy as np

import concourse.bass as bass
import concourse.bacc as bacc
import concourse.mybir as mybir
import concourse.tile as tile
from concourse import bass_utils

F32 = mybir.dt.float32
F32R = mybir.dt.float32r
U32 = mybir.dt.uint32
DVE = mybir.EngineType.DVE
ACT = mybir.EngineType.Activation
PE = mybir.EngineType.PE
SP = mybir.EngineType.SP
AF = mybir.ActivationFunctionType
OP = mybir.AluOpType

T_FULL = 16384
N_DIM = 256
M_DIM = 512     # hidden size; [128, 4] layout
KA = 512        # number of addresses (K+1)
L_OUT = 256
G3 = 3 * M_DIM  # 1536; [128, 12] layout


GAP_TAU_BITS = 0x3C23D70A  # float bits of 0.01: narrow-gap threshold


def build_nc(T=T_FULL, CH=512, loops=1):
    assert T % CH == 0
    NCH = T // CH
    nc = bacc.Bacc("TRN2", target_bir_lowering=False, debug=False,
                   enable_asserts=False)

    X_d = nc.dram_tensor("X", [T, N_DIM], F32, kind="ExternalInput")
    h0_d = nc.dram_tensor("h0", [M_DIM], F32, kind="ExternalInput")
    Wih_d = nc.dram_tensor("W_ih", [G3, N_DIM], F32, kind="ExternalInput")
    Whh_d = nc.dram_tensor("W_hh", [G3, M_DIM], F32, kind="ExternalInput")
    bih_d = nc.dram_tensor("b_ih", [G3], F32, kind="ExternalInput")
    bhh_d = nc.dram_tensor("b_hh", [G3], F32, kind="ExternalInput")
    Cw_d = nc.dram_tensor("C_w", [KA, M_DIM], F32, kind="ExternalInput")
    Cb_d = nc.dram_tensor("C_b", [KA], F32, kind="ExternalInput")
    Vw_d = nc.dram_tensor("V_w", [L_OUT, M_DIM], F32, kind="ExternalInput")
    Vb_d = nc.dram_tensor("V_b", [L_OUT], F32, kind="ExternalInput")
    Y_d = nc.dram_tensor("Y", [T, L_OUT], F32, kind="ExternalOutput")

    GiT_d = nc.dram_tensor("GiT", [12, 128, T], F32, kind="Internal")
    Ht_d = nc.dram_tensor("Ht", [M_DIM, T], F32, kind="Internal")

    with tile.TileContext(nc) as tc:
        with (
            tc.tile_pool(name="state", bufs=1) as st,
            tc.tile_pool(name="scratch", bufs=1) as sc,
            tc.tile_pool(name="psum", bufs=1, space="PSUM") as pp,
            tc.tile_pool(name="psum2", bufs=2, space="PSUM") as pp2,
        ):
            # ---- persistent state / weights in SBUF ----
            WihT = st.tile([128, 24, 128], F32)      # (kp, k*12+j, m)
            WhhT = st.tile([128, 48, 128], F32)     # (dp, k*12+j, m)
            CwT = st.tile([128, 4, KA], F32)         # (dp, k, a)
            VwT = st.tile([128, 4, L_OUT], F32)    # (dp, k, l)
            bb_row = st.tile([1, G3], F32)           # b_ih + b_hh (r,z only)
            bhh_row = st.tile([1, G3], F32)
            bhh_nT = st.tile([1, M_DIM], F32)        # b_hn row (for GHtab)
            Cb_row = st.tile([1, KA], F32)
            Vb_row = st.tile([1, L_OUT], F32)
            ones_row = st.tile([1, 512], F32)
            one1 = st.tile([1, 1], F32)
            POS = st.tile([1, KA], U32)             # [0,1,1,...,1]

            GHtab = st.tile([128, 12, KA], F32)     # memoized W_hh@M[q]
            Mtab = st.tile([128, 4, KA], F32)       # memory rows
            wflag = st.tile([1, KA], U32)           # written flags
            Hbuf = st.tile([128, CH + 1, 4], F32)   # h_out history (chunk)
            giT = st.tile([128, 12, CH], F32)       # streamed Gi chunk
            idx8 = st.tile([1, 8], U32)
            mx8 = st.tile([1, 8], F32)


            g8 = sc.tile([128, 8], F32)             # i_rz + h_rz
            rz = sc.tile([128, 8], F32)             # sigmoid out
            t4 = sc.tile([128, 4], F32)
            u4 = sc.tile([128, 4], F32)
            ng = sc.tile([128, 4], F32)
            d4 = sc.tile([128, 4], F32)
            e4 = sc.tile([128, 4], F32)
            hnew = sc.tile([128, 4], F32)

            lg_ps = pp.tile([1, KA], F32)
            gh_ps = pp.tile([128, 12], F32)

            # ---- load weights (one-time) ----
            for j in range(12):
                for k in range(2):
                    nc.sync.dma_start(
                        WihT[:, k * 12 + j, :],
                        Wih_d[128 * j:128 * (j + 1),
                              128 * k:128 * (k + 1)].transpose([1, 0]))
                for k in range(4):
                    nc.sync.dma_start(
                        WhhT[:, k * 12 + j, :],
                        Whh_d[128 * j:128 * (j + 1),
                              128 * k:128 * (k + 1)].transpose([1, 0]))
            for k in range(4):
                nc.sync.dma_start(
                    CwT[:, k, :], Cw_d[:, 128 * k:128 * (k + 1)].transpose([1, 0]))
                nc.sync.dma_start(
                    VwT[:, k, :], Vw_d[:, 128 * k:128 * (k + 1)].transpose([1, 0]))
            nc.sync.dma_start(bb_row[:], bih_d[None, :])
            nc.sync.dma_start(bhh_row[:], bhh_d[None, :])
            nc.sync.dma_start(Cb_row[:], Cb_d[None, :])
            nc.sync.dma_start(Vb_row[:], Vb_d[None, :])
            nc.sync.dma_start(bhh_nT[:], bhh_d[None, 2 * M_DIM:3 * M_DIM])
            # fold b_hh into Gi for the r,z gates only; b_hn lives in GHtab
            # (torch GRU: n = tanh(W_in x + b_in + r*(W_hn h + b_hn)))
            nc.vector.tensor_tensor(bb_row[0:1, 0:2 * M_DIM],
                                    bb_row[0:1, 0:2 * M_DIM],
                                    bhh_row[0:1, 0:2 * M_DIM], OP.add)
            nc.vector.memset(ones_row[:], 1.0)
            nc.vector.memset(one1[:], 1.0)
            nc.vector.memset(wflag[:], 0)
            nc.vector.memset(POS[:], 1)
            nc.vector.memset(POS[0:1, 0:1], 0)
            nc.vector.memset(idx8[:], 0)
            nc.vector.memset(Mtab[:], 0.0)
            nc.vector.memset(GHtab[:], 0.0)
            nc.vector.memset(Hbuf[:], 0.0)
            # h_prev is always read from Mtab[q]; q starts at 0
            nc.sync.dma_start(Mtab[:, :, 0],
                              h0_d.rearrange("(j p) -> p j", p=128))

            # prime GHtab[0] = W_hh @ h0 (+ b_hn on the n section)
            h0_sb = sc.tile([128, 4], F32)
            nc.sync.dma_start(h0_sb[:], h0_d.rearrange("(j p) -> p j", p=128))

            def gh_matmuls(hsrc):
                for j in range(12):
                    for k in range(4):
                        nc.tensor.matmul(gh_ps[:, j:j + 1],
                                         WhhT[:, k * 12 + j, :],
                                         hsrc[:, k:k + 1],
                                         start=(k == 0),
                                         stop=(k == 3 and j < 8))
                    if j >= 8:
                        nc.tensor.matmul(gh_ps[:, j:j + 1],
                                         bhh_nT[0:1, 128 * (j - 8):128 * (j - 7)],
                                         one1[:], start=False, stop=True)

            gh_matmuls(h0_sb)
            nc.vector.tensor_copy(GHtab[:, :, 0], gh_ps[:])

            # ---- phase 1: GiT = (X @ W_ih.T + b_ih + b_hh) as [12,128,T]
            P1C = min(512, T)
            assert T % P1C == 0
            for c in range(T // P1C):
                xts = []
                for half in range(2):
                    xt = sc.tile([128, P1C], F32, tag=f"xt{half}")
                    nc.sync.dma_start(
                        xt[:], X_d[P1C * c:P1C * (c + 1),
                                   128 * half:128 * (half + 1)].transpose([1, 0]))
                    xts.append(xt)
                for j in range(12):
                    ps = pp2.tile([128, P1C], F32, tag="p1ps")
                    nc.tensor.matmul(ps[:], bb_row[0:1, 128 * j:128 * (j + 1)],
                                     ones_row[0:1, 0:P1C],
                                     start=True, stop=False)
                    for k in range(2):
                        nc.tensor.matmul(ps[:], WihT[:, k * 12 + j, :],
                                         xts[k][:], start=False, stop=(k == 1))
                    gi_out = sc.tile([128, P1C], F32, tag="giout")
                    nc.scalar.activation(gi_out[:], ps[:], AF.Copy)
                    nc.sync.dma_start(GiT_d[j, :, P1C * c:P1C * (c + 1)],
                                      gi_out[:])

            # ---- phase 2: the recurrence ----
            Gi_v = GiT_d.rearrange("j p t -> p j t")

            def load_q(engines):
                return nc.values_load(
                    idx8[0:1, 0:1], engines=engines,
                    min_val=0, max_val=KA - 1,
                    skip_runtime_bounds_check=True)

            with tc.For_i(0, NCH * loops,
                          hint_engines=(PE, DVE, ACT, SP)) as ch_raw:
                ch = (ch_raw % NCH) if loops > 1 else ch_raw
                nc.sync.dma_start(giT[:], Gi_v[:, :, bass.ds(ch * CH, CH)])
                q = load_q([DVE, ACT, PE])

                for i in range(CH):
                    # C_b bias matmul can start before gates finish
                    nc.tensor.matmul(lg_ps[:], one1[:], Cb_row[:],
                                     start=True, stop=False)
                    # gates: r,z parts
                    nc.vector.tensor_tensor(
                        g8[:], giT[:, 0:8, bass.ds(i, 1)],
                        GHtab[:, 0:8, bass.ds(q, 1)], OP.add)
                    nc.scalar.activation(rz[:], g8[:], AF.Sigmoid)
                    # n part: tanh(gi_n + r * gh_n)
                    nc.vector.tensor_tensor(t4[:], rz[:, 0:4],
                                            GHtab[:, 8:12, bass.ds(q, 1)], OP.mult)
                    nc.vector.tensor_tensor(u4[:], t4[:],
                                            giT[:, 8:12, bass.ds(i, 1)], OP.add)
                    nc.scalar.activation(ng[:], u4[:], AF.Tanh)
                    # h_new = ng + z*(h_prev - ng); h_prev == Mtab[q_prev]
                    nc.vector.tensor_tensor(d4[:], Mtab[:, :, bass.ds(q, 1)],
                                            ng[:], OP.subtract)
                    nc.vector.tensor_tensor(e4[:], rz[:, 4:8], d4[:], OP.mult)
                    nc.vector.tensor_tensor(hnew[:], ng[:], e4[:], OP.add)
                    # logits += C_w @ h_new
                    for k in range(4):
                        nc.tensor.matmul(lg_ps[:], hnew[:, k:k + 1],
                                         CwT[:, k, :],
                                         start=False, stop=(k == 3))
                    # argmax straight from PSUM
                    nc.vector.max(mx8[:], lg_ps[:])
                    nc.vector.max_index(idx8[:], mx8[:], lg_ps[:])
                    q2 = load_q([DVE, ACT, PE])
                    f = nc.values_load(wflag[0:1, bass.ds(q2, 1)],
                                       engines=[ACT, PE],
                                       skip_runtime_bounds_check=True)
                    with tc.If(f == 0):
                        # miss: write-once M/GHtab update (ACT+PE only)
                        nc.scalar.copy(Mtab[:, :, bass.ds(q2, 1)], hnew[:])
                        nc.scalar.copy(wflag[0:1, bass.ds(q2, 1)],
                                       POS[0:1, bass.ds(q2, 1)])
                        gh_matmuls(hnew)
                        nc.scalar.copy(GHtab[:, :, bass.ds(q2, 1)], gh_ps[:])
                    # h_out == Mtab[q2] on both paths now
                    nc.scalar.copy(Hbuf[:, i + 1, :],
                                   Mtab[:, :, bass.ds(q2, 1)])
                    q = q2

                for j in range(4):
                    nc.sync.dma_start(
                        Ht_d[128 * j:128 * (j + 1), bass.ds(ch * CH, CH)],
                        Hbuf[:, 1:CH + 1, j])

            # ---- phase 3: Y = H @ V_w.T + V_b ----
            for tt in range(T // 128):
                ps = pp2.tile([128, L_OUT], F32, tag="p3ps")
                nc.tensor.matmul(ps[:], ones_row[0:1, 0:128], Vb_row[:],
                                 start=True, stop=False)
                for k in range(4):
                    hts = sc.tile([128, 128], F32, tag="hts")
                    nc.sync.dma_start(
                        hts[:], Ht_d[128 * k:128 * (k + 1),
                                     128 * tt:128 * (tt + 1)])
                    nc.tensor.matmul(ps[:], hts[:], VwT[:, k, :],
                                     start=False, stop=(k == 3))
                y_out = sc.tile([128, L_OUT], F32, tag="yout")
                nc.vector.tensor_copy(y_out[:], ps[:])
                nc.sync.dma_start(Y_d[128 * tt:128 * (tt + 1), :], y_out[:])

    nc.compile()
    return nc


_NC_CACHE = {}


def _get_nc(T=T_FULL, CH=512):
    key = (T, CH)
    if key not in _NC_CACHE:
        _NC_CACHE[key] = build_nc(T, CH)
    return _NC_CACHE[key]


def kernel(**inputs):
    nc = _get_nc()
    in_map = {k: np.ascontiguousarray(np.asarray(v, np.float32))
              for k, v in inputs.items()}
    res = bass_utils.run_bass_kernel_spmd(nc, [in_map] * 8,
                                          core_ids=list(range(8)))
    return res.results[0]["Y"]

